# revision 1
# baseline (speedup 1.0000x reference)
"""Trainium2 Bass kernel for single-head attention (MDTA-style block).

Reference computation (per batch b, N=4096 tokens, C=128 channels):
    qkv = x @ W_fc + b_fc ; q,k,v = split(qkv)
    S   = (q @ k^T) / sqrt(C)
    A   = softmax(S / scale, axis=-1)
    out = (A @ v) @ W_out + b_out

Sharding: 8 cores = 4 batches x 2 query-halves (data parallel, no
cross-core comm). Each core computes 2048 query rows against the full
4096 keys/values of its batch.

Per-core device algorithm (flash-style, NxN never hits HBM):
  - qkv projection from x^T (C on partitions) via PE matmuls:
      q^T, k^T in [C, n] layout; v in natural [m, C] tiles (bf16).
  - scores computed TRANSPOSED: S^T[m, n] = k^T.T @ q^T, per 128-key tile,
    512-query block -> PSUM.
  - exp on ScalarE (fused 1/(sqrt(C)*scale) scaling; max-subtraction is
    skipped: scores are O(+-6) so exp is fp32-safe; softmax is shift-
    invariant so the result is identical).
  - A^T tiles (bf16) feed two PE accumulation chains: out_u^T = v^T-
    weighted sum, and row-sums via a ones-vector matmul.
  - normalize with VectorE reciprocal + partition-broadcast multiply,
    project with W_out, add (b_v @ W_out + b_out) (folded on host).
"""

import math
import sys

import numpy as np

sys.path.insert(0, "/opt/trn_rl_repo")

import ml_dtypes  # noqa: E402

import concourse.bacc as bacc  # noqa: E402
import concourse.mybir as mybir  # noqa: E402
import concourse.tile as tile  # noqa: E402
from concourse.bass_utils import run_bass_kernel_spmd  # noqa: E402

B, N, C = 4, 4096, 128
NCORES = 8
NQ = N // 2  # queries per core
NB = 512  # query block size
NMT = N // C  # key tiles (32)
F32 = mybir.dt.float32
F32R = mybir.dt.float32r
BF16 = mybir.dt.bfloat16

_cache: dict = {}
LAST_RESULTS = None


def _build(sc: float):
    nc = bacc.Bacc(None, target_bir_lowering=False, debug=True)

    xT = nc.declare_dram_parameter("xT", [C, N], F32, isOutput=False)
    xTq = nc.declare_dram_parameter("xTq", [C, NQ], F32, isOutput=False)
    Wq = nc.declare_dram_parameter("Wq", [C, C], F32, isOutput=False)
    Wk = nc.declare_dram_parameter("Wk", [C, C], F32, isOutput=False)
    Wv = nc.declare_dram_parameter("Wv", [C, C], F32, isOutput=False)
    Wo = nc.declare_dram_parameter("Wo", [C, C], F32, isOutput=False)
    bq = nc.declare_dram_parameter("bq", [C, 1], F32, isOutput=False)
    bk = nc.declare_dram_parameter("bk", [C, 1], F32, isOutput=False)
    b2b = nc.declare_dram_parameter("b2b", [C, C], F32, isOutput=False)
    ones = nc.declare_dram_parameter("ones", [C, C], BF16, isOutput=False)
    y = nc.declare_dram_parameter("y", [NQ, C], F32, isOutput=True)

    with tile.TileContext(nc) as tc:
        with (
            tc.tile_pool(name="const", bufs=1) as cp,
            tc.tile_pool(name="ebuf", bufs=2) as ep,
            tc.tile_pool(name="obuf", bufs=2) as op,
            tc.tile_pool(name="ybuf", bufs=3) as yp,
            tc.tile_pool(name="small", bufs=2) as sp,
            tc.tile_pool(name="ps_s", bufs=2, space="PSUM") as ps_s,
            tc.tile_pool(name="ps_o", bufs=2, space="PSUM") as ps_o,
            tc.tile_pool(name="ps_sum", bufs=1, space="PSUM") as ps_sum,
            tc.tile_pool(name="ps_p", bufs=2, space="PSUM") as ps_p,
        ):
            xT_s = cp.tile([C, N], F32)
            xTq_s = cp.tile([C, NQ], F32)
            wq_s = cp.tile([C, C], F32)
            wk_s = cp.tile([C, C], F32)
            wv_s = cp.tile([C, C], F32)
            wo_s = cp.tile([C, C], F32)
            bq_s = cp.tile([C, 1], F32)
            bk_s = cp.tile([C, 1], F32)
            b2b_s = cp.tile([C, C], F32)
            ones_s = cp.tile([C, C], BF16)
            kT_s = cp.tile([C, N], BF16)
            qT_s = cp.tile([C, NQ], BF16)
            v_s = cp.tile([C, N], BF16)

            for dst, src in [
                (xT_s, xT), (xTq_s, xTq), (wq_s, Wq), (wk_s, Wk),
                (wv_s, Wv), (wo_s, Wo), (bq_s, bq), (bk_s, bk),
                (b2b_s, b2b), (ones_s, ones),
            ]:
                nc.sync.dma_start(out=dst[:], in_=src[:])

            # k^T = (Wk.T @ x^T) + bk ; q^T likewise
            for ch in range(N // NB):
                sl = slice(ch * NB, (ch + 1) * NB)
                ps = ps_s.tile([C, NB], F32, tag="pss")
                nc.tensor.matmul(
                    ps[:], wk_s[:], xT_s[:, sl],
                    start=True, stop=True,
                )
                nc.vector.tensor_scalar_add(kT_s[:, sl], ps[:], bk_s[:])
            for ch in range(NQ // NB):
                sl = slice(ch * NB, (ch + 1) * NB)
                ps = ps_s.tile([C, NB], F32, tag="pss")
                nc.tensor.matmul(
                    ps[:], wq_s[:], xTq_s[:, sl],
                    start=True, stop=True,
                )
                nc.vector.tensor_scalar_add(qT_s[:, sl], ps[:], bq_s[:])
            # v tiles in natural [m, C] layout (m on partitions), bf16
            for mt in range(NMT):
                sl = slice(mt * C, (mt + 1) * C)
                psv = ps_p.tile([C, C], F32, tag="psp")
                nc.tensor.matmul(
                    psv[:], xT_s[:, sl], wv_s[:],
                    start=True, stop=True,
                )
                nc.vector.tensor_copy(v_s[:, sl], psv[:])

            for nb in range(NQ // NB):
                qsl = slice(nb * NB, (nb + 1) * NB)
                E = ep.tile([C, NMT * NB], BF16, tag="E")
                pso = ps_o.tile([C, NB], F32, tag="pso")
                pssum = ps_sum.tile([C, NB], F32, tag="pssum")
                for mt in range(NMT):
                    msl = slice(mt * C, (mt + 1) * C)
                    esl = slice(mt * NB, (mt + 1) * NB)
                    pss = ps_s.tile([C, NB], F32, tag="pss")
                    nc.tensor.matmul(
                        pss[:], kT_s[:, msl],
                        qT_s[:, qsl], start=True, stop=True,
                    )
                    nc.scalar.activation(
                        E[:, esl], pss[:], mybir.ActivationFunctionType.Exp,
                        scale=sc,
                    )
                    nc.tensor.matmul(
                        pso[:], v_s[:, msl], E[:, esl],
                        start=(mt == 0), stop=(mt == NMT - 1),
                    )
                    nc.tensor.matmul(
                        pssum[:], ones_s[:], E[:, esl],
                        start=(mt == 0), stop=(mt == NMT - 1),
                    )
                rcp_b = sp.tile([C, NB], F32, tag="rcpb")
                nc.vector.reciprocal(rcp_b[:], pssum[:])
                oT = op.tile([C, NB], F32, tag="oT")
                nc.vector.tensor_tensor(
                    oT[:], pso[:], rcp_b[:], op=mybir.AluOpType.mult,
                )
                for k in range(NB // C):
                    osl = slice(k * C, (k + 1) * C)
                    psp = ps_p.tile([C, C], F32, tag="psp")
                    nc.tensor.matmul(
                        psp[:], oT[:, osl], wo_s[:],
                        start=True, stop=True,
                    )
                    yt = yp.tile([C, C], F32, tag="yt")
                    nc.vector.tensor_add(yt[:], psp[:], b2b_s[:])
                    nc.sync.dma_start(
                        out=y[nb * NB + k * C: nb * NB + (k + 1) * C, :],
                        in_=yt[:],
                    )

    nc.compile()
    return nc


def kernel(x, W_fc, b_fc, W_out, b_out, scale):
    x = np.asarray(x, dtype=np.float32)
    W_fc = np.asarray(W_fc, dtype=np.float32)
    b_fc = np.asarray(b_fc, dtype=np.float32)
    W_out = np.asarray(W_out, dtype=np.float32)
    b_out = np.asarray(b_out, dtype=np.float32)
    scale = np.asarray(scale, dtype=np.float32)

    sc = float(1.0 / (math.sqrt(C) * float(scale[0])))
    key = ("v1", sc)
    if key not in _cache:
        _cache.clear()
        _cache[key] = _build(sc)
    nc = _cache[key]

    b2 = b_fc[2 * C:] @ W_out + b_out  # fold v-bias through the projection
    common = {
        "Wq": np.ascontiguousarray(W_fc[:, :C]),
        "Wk": np.ascontiguousarray(W_fc[:, C:2 * C]),
        "Wv": np.ascontiguousarray(W_fc[:, 2 * C:]),
        "Wo": W_out,
        "bq": np.ascontiguousarray(b_fc[:C].reshape(C, 1)),
        "bk": np.ascontiguousarray(b_fc[C:2 * C].reshape(C, 1)),
        "b2b": np.ascontiguousarray(np.tile(b2, (C, 1))),
        "ones": np.ones((C, C), dtype=ml_dtypes.bfloat16),
    }
    in_maps = []
    for core in range(NCORES):
        b, h = core // 2, core % 2
        xT_b = np.ascontiguousarray(x[b].T)
        xTq_b = np.ascontiguousarray(x[b, h * NQ:(h + 1) * NQ, :].T)
        in_maps.append({**common, "xT": xT_b, "xTq": xTq_b})

    res = run_bass_kernel_spmd(nc, in_maps, list(range(NCORES)))
    global LAST_RESULTS
    LAST_RESULTS = res

    y = np.empty((B, N, C), dtype=np.float32)
    for core in range(NCORES):
        b, h = core // 2, core % 2
        y[b, h * NQ:(h + 1) * NQ, :] = res.results[core]["y"]
    return y



# revision 6
# speedup vs baseline: 1.5542x; 1.5542x over previous
"""Trainium2 Bass kernel for single-head attention (MDTA-style block).

Reference computation (per batch b, N=4096 tokens, C=128 channels):
    qkv = x @ W_fc + b_fc ; q,k,v = split(qkv)
    S   = (q @ k^T) / sqrt(C)
    A   = softmax(S / scale, axis=-1)
    out = (A @ v) @ W_out + b_out

Sharding: 8 cores = 4 batches x 2 query-halves (data parallel, no
cross-core comm). Each core computes 2048 query rows against the full
4096 keys/values of its batch.

Per-core algorithm (v2 — flash-style, NxN never hits HBM):
  - q^T, k^T projections in [C, n] layout (bf16 PE matmuls from x^T).
    k-bias dropped (softmax shift-invariance), q-bias kept.
  - scores TRANSPOSED per 128-key tile: S^T[k,q] = kT.T @ qT into a
    [128, 1024] PSUM pair (2 key tiles), exp'd by ONE ScalarE
    activation (wide tiles amortize the ~172cyc PSUM latency).
  - value path via the identity (A@V)@Wo = (A@X) @ (Wv@Wo):
    Z = sum_mt x_mt^T @ E_mt accumulates with RAW x tiles stationary
    (no v projection at all); per 128-query chunk one matmul
    Z_chunk^T @ Wvo lands the result directly in [q, C] row layout.
  - softmax denominator: E pair-tiles are accumulated on VectorE into
    two bf16 partial sums (halves rounding error, breaks the serial
    chain); tiny matmuls Esum_chunk^T @ ones give rowsums TRANSPOSED
    [q, 1], so the (8x slow) reciprocal runs on FD=4 only and the
    final normalize is a per-partition scalar fused with the bias add
    (one scalar_tensor_tensor per chunk).
  - block tails are emitted two pair-iterations into the next block so
    the PE/ACT streams never drain at block boundaries.
"""

import math
import sys

import numpy as np

sys.path.insert(0, "/opt/trn_rl_repo")

import ml_dtypes  # noqa: E402

import concourse.bacc as bacc  # noqa: E402
import concourse.mybir as mybir  # noqa: E402
import concourse.tile as tile  # noqa: E402
from concourse.bass_utils import run_bass_kernel_spmd  # noqa: E402

B, N, C = 4, 4096, 128
NCORES = 8
NQ = N // 2  # queries per core
NB = 512  # query block size
NMT = N // C  # key tiles (32)
F32 = mybir.dt.float32
BF16 = mybir.dt.bfloat16
AOP = mybir.AluOpType

_cache: dict = {}
LAST_RESULTS = None


def _build(sc: float):
    """Build the per-core kernel. Queries are always tokens 0..NQ-1 of the
    (host-rotated) token axis, so one SPMD program serves both halves."""
    nc = bacc.Bacc(None, target_bir_lowering=False, debug=True)

    xT = nc.declare_dram_parameter("xT", [C, N], BF16, isOutput=False)
    xN = nc.declare_dram_parameter("xN", [C, NMT * C], BF16, isOutput=False)
    Wq = nc.declare_dram_parameter("Wq", [C, C], BF16, isOutput=False)
    Wk = nc.declare_dram_parameter("Wk", [C, C], BF16, isOutput=False)
    Wvo = nc.declare_dram_parameter("Wvo", [C, C], BF16, isOutput=False)
    bq = nc.declare_dram_parameter("bq", [C, 1], F32, isOutput=False)
    b2b = nc.declare_dram_parameter("b2b", [C, C], F32, isOutput=False)
    ones = nc.declare_dram_parameter("ones", [C, 1], BF16, isOutput=False)
    y = nc.declare_dram_parameter("y", [NQ, C], F32, isOutput=True)

    qoff = 0  # queries live in the first NQ token columns
    # token chunks of 512 stream in natural order = query chunks first
    chunk_order = list(range(8))
    mt_order = list(range(NMT))

    with tile.TileContext(nc) as tc:
        with (
            tc.tile_pool(name="const", bufs=1) as cp,
            tc.tile_pool(name="ebuf", bufs=3) as ep,
            tc.tile_pool(name="esum", bufs=2) as esp,
            tc.tile_pool(name="small", bufs=2) as sp,
            tc.tile_pool(name="ybuf", bufs=3) as yp,
            tc.tile_pool(name="ps_s", bufs=2, space="PSUM") as ps_s,
            tc.tile_pool(name="ps_z", bufs=2, space="PSUM") as ps_z,
            tc.tile_pool(name="ps_rs", bufs=1, space="PSUM") as ps_rs,
            tc.tile_pool(name="ps_y", bufs=1, space="PSUM") as ps_y,
        ):
            xT_s = cp.tile([C, N], BF16)
            xN_s = cp.tile([C, NMT * C], BF16)
            wq_s = cp.tile([C, C], BF16)
            wk_s = cp.tile([C, C], BF16)
            wvo_s = cp.tile([C, C], BF16)
            bq_s = cp.tile([C, 1], F32)
            b2b_s = cp.tile([C, C], F32)
            ones_s = cp.tile([C, 1], BF16)
            kT_s = cp.tile([C, N], BF16)
            qT_s = cp.tile([C, NQ], BF16)

            for dst, src in [
                (wq_s, Wq), (wk_s, Wk), (wvo_s, Wvo),
                (bq_s, bq), (b2b_s, b2b), (ones_s, ones),
            ]:
                nc.sync.dma_start(out=dst[:], in_=src[:])
            for c in chunk_order:
                sl = slice(c * NB, (c + 1) * NB)
                nc.sync.dma_start(out=xT_s[:, sl], in_=xT[:, sl])
                nc.sync.dma_start(out=xN_s[:, sl], in_=xN[:, sl])

            # q^T = Wq.T @ xT[:, qcols] + bq  (pairs of 512 per PSUM slot)
            for j in range(2):
                ps = ps_s.tile([C, 2 * NB], F32, tag="s")
                for g in range(2):
                    o = qoff + (2 * j + g) * NB
                    nc.tensor.matmul(
                        ps[:, g * NB:(g + 1) * NB], wq_s[:],
                        xT_s[:, o:o + NB], start=True, stop=True,
                    )
                nc.vector.tensor_scalar_add(
                    qT_s[:, 2 * j * NB:(2 * j + 2) * NB], ps[:], bq_s[:],
                )
            # k^T = Wk.T @ xT (k-bias cancels in softmax)
            for j in range(4):
                ps = ps_s.tile([C, 2 * NB], F32, tag="s")
                for g in range(2):
                    c = chunk_order[2 * j + g]
                    nc.tensor.matmul(
                        ps[:, g * NB:(g + 1) * NB], wk_s[:],
                        xT_s[:, c * NB:(c + 1) * NB], start=True, stop=True,
                    )
                for g in range(2):
                    c = chunk_order[2 * j + g]
                    nc.vector.tensor_copy(
                        kT_s[:, c * NB:(c + 1) * NB],
                        ps[:, g * NB:(g + 1) * NB],
                    )

            pending_tail = [None]

            def make_tail(nb, esA, esB, z_ps):
                def emit():
                    zT = sp.tile([C, NB], BF16, tag="zT")
                    nc.vector.tensor_copy(zT[:], z_ps[:])
                    rs = ps_rs.tile([C, 4], F32, tag="rs")
                    for j in range(4):
                        csl = slice(j * C, (j + 1) * C)
                        nc.tensor.matmul(
                            rs[:, j:j + 1], esA[:, csl], ones_s[:],
                            start=True, stop=False,
                        )
                        nc.tensor.matmul(
                            rs[:, j:j + 1], esB[:, csl], ones_s[:],
                            start=False, stop=True,
                        )
                    rcp = sp.tile([C, 4], F32, tag="rcp")
                    nc.vector.reciprocal(rcp[:], rs[:])
                    for j in range(4):
                        csl = slice(j * C, (j + 1) * C)
                        pp = ps_y.tile([C, C], F32, tag="yp")
                        nc.tensor.matmul(
                            pp[:], zT[:, csl], wvo_s[:],
                            start=True, stop=True,
                        )
                        yt = yp.tile([C, C], F32, tag="yt")
                        nc.vector.scalar_tensor_tensor(
                            yt[:], pp[:], rcp[:, j:j + 1], b2b_s[:],
                            op0=AOP.mult, op1=AOP.add,
                        )
                        r0 = nb * NB + j * C
                        nc.sync.dma_start(out=y[r0:r0 + C, :], in_=yt[:])
                return emit

            for nb in range(NQ // NB):
                qsl = slice(nb * NB, (nb + 1) * NB)
                z_ps = ps_z.tile([C, NB], F32, tag="z")
                esA = esp.tile([C, NB], BF16, tag="esA")
                esB = esp.tile([C, NB], BF16, tag="esB")
                for mp in range(NMT // 2):
                    mta = mt_order[2 * mp]
                    mtb = mt_order[2 * mp + 1]
                    asl = slice(mta * C, (mta + 1) * C)
                    bsl = slice(mtb * C, (mtb + 1) * C)
                    pss = ps_s.tile([128, 2 * NB], F32, tag="s")
                    nc.tensor.matmul(
                        pss[:, 0:NB], kT_s[:, asl], qT_s[:, qsl],
                        start=True, stop=True,
                    )
                    nc.tensor.matmul(
                        pss[:, NB:2 * NB], kT_s[:, bsl], qT_s[:, qsl],
                        start=True, stop=True,
                    )
                    if mp == 2 and pending_tail[0] is not None:
                        pending_tail[0]()
                        pending_tail[0] = None
                    E = ep.tile([128, 2 * NB], BF16, tag="E")
                    nc.scalar.activation(
                        E[:], pss[:], mybir.ActivationFunctionType.Exp,
                        scale=sc,
                    )
                    nc.tensor.matmul(
                        z_ps[:], xN_s[:, asl], E[:, 0:NB],
                        start=(mp == 0), stop=False,
                    )
                    nc.tensor.matmul(
                        z_ps[:], xN_s[:, bsl], E[:, NB:2 * NB],
                        start=False, stop=(mp == NMT // 2 - 1),
                    )
                    if mp == 0:
                        nc.vector.tensor_tensor(
                            esA[:], E[:, 0:NB], E[:, NB:2 * NB], op=AOP.add,
                        )
                    elif mp == 1:
                        nc.vector.tensor_tensor(
                            esB[:], E[:, 0:NB], E[:, NB:2 * NB], op=AOP.add,
                        )
                    else:
                        acc = esA if mp % 2 == 0 else esB
                        nc.vector.tensor_tensor(
                            acc[:], acc[:], E[:, 0:NB], op=AOP.add,
                        )
                        nc.vector.tensor_tensor(
                            acc[:], acc[:], E[:, NB:2 * NB], op=AOP.add,
                        )
                pending_tail[0] = make_tail(nb, esA, esB, z_ps)
            pending_tail[0]()

    nc.compile()
    return nc


def kernel(x, W_fc, b_fc, W_out, b_out, scale):
    x = np.asarray(x, dtype=np.float32)
    W_fc = np.asarray(W_fc, dtype=np.float32)
    b_fc = np.asarray(b_fc, dtype=np.float32)
    W_out = np.asarray(W_out, dtype=np.float32)
    b_out = np.asarray(b_out, dtype=np.float32)
    scale = np.asarray(scale, dtype=np.float32)

    sc = float(1.0 / (math.sqrt(C) * float(scale[0])))
    key = ("v2", sc)
    if key not in _cache:
        _cache.clear()
        _cache[key] = _build(sc)
    nc = _cache[key]

    bf16 = ml_dtypes.bfloat16
    b2 = b_fc[2 * C:] @ W_out + b_out  # v-bias folded through the projection
    common = {
        "Wq": np.ascontiguousarray(W_fc[:, :C]).astype(bf16),
        "Wk": np.ascontiguousarray(W_fc[:, C:2 * C]).astype(bf16),
        "Wvo": np.ascontiguousarray(W_fc[:, 2 * C:] @ W_out).astype(bf16),
        "bq": np.ascontiguousarray(b_fc[:C].reshape(C, 1)),
        "b2b": np.ascontiguousarray(np.tile(b2, (C, 1))),
        "ones": np.ones((C, 1), dtype=bf16),
    }
    in_maps = []
    for core in range(NCORES):
        b, h = core // 2, core % 2
        # rotate tokens so this core's queries are rows 0..NQ-1 (key order
        # inside the softmax sum is irrelevant)
        xb = np.roll(x[b], -h * NQ, axis=0) if h else x[b]
        xT_b = np.ascontiguousarray(xb.T).astype(bf16)
        # partition-major tiling: xN[p, mt*C + j] = x[mt*128 + p, j]
        xN_b = np.ascontiguousarray(
            xb.reshape(NMT, C, C).transpose(1, 0, 2).reshape(C, NMT * C)
        ).astype(bf16)
        in_maps.append({**common, "xT": xT_b, "xN": xN_b})

    res = run_bass_kernel_spmd(nc, in_maps, list(range(NCORES)))
    global LAST_RESULTS
    LAST_RESULTS = res

    yout = np.empty((B, N, C), dtype=np.float32)
    for core in range(NCORES):
        b, h = core // 2, core % 2
        yout[b, h * NQ:(h + 1) * NQ, :] = res.results[core]["y"]
    return yout


# revision 8
# speedup vs baseline: 1.5673x; 1.0084x over previous
"""Trainium2 Bass kernel for single-head attention (MDTA-style block).

Reference computation (per batch b, N=4096 tokens, C=128 channels):
    qkv = x @ W_fc + b_fc ; q,k,v = split(qkv)
    S   = (q @ k^T) / sqrt(C)
    A   = softmax(S / scale, axis=-1)
    out = (A @ v) @ W_out + b_out

Sharding: 8 cores = 4 batches x 2 query-halves (data parallel, no
cross-core comm). The token axis is rotated host-side for odd cores so
one SPMD program always sees its queries as tokens 0..2047 (softmax is
invariant to key order).

Per-core algorithm (v3 — flash-style, NxN never hits HBM):
  - q^T, k^T projections in [C, n] layout (bf16 PE matmuls from x^T).
    k-bias dropped (softmax shift-invariance), q-bias kept.
  - scores TRANSPOSED per 128-key tile: S^T[k,q] = kT.T @ qT into a
    [128, 1024] PSUM pair (2 key tiles), exp'd by ONE ScalarE
    activation (wide tiles amortize the ~172cyc PSUM latency).
  - value path via the identity (A@V)@Wo = (A@X) @ (Wv@Wo):
    Z = sum_mt x_mt^T @ E_mt accumulates with RAW x tiles stationary
    (no v projection at all); per 128-query chunk one matmul
    Z_chunk^T @ Wvo lands the result directly in [q, C] row layout.
  - softmax denominator: E pair-tiles accumulate on VectorE into one
    [128,1024] bf16 running sum (one wide add per pair), folded once;
    tiny matmuls esF_chunk^T @ ones give rowsums TRANSPOSED [q, 1] so
    the (8x slow) reciprocal runs on FD=4 only and the normalize is a
    per-partition scalar fused with the bias add.
  - block tails are emitted two pair-iterations into the next block so
    the PE/ACT streams never drain at block boundaries; input DMAs are
    batched (6 total) and split across the Sync and Scalar queues.
"""

import math
import sys

import numpy as np

sys.path.insert(0, "/opt/trn_rl_repo")

import ml_dtypes  # noqa: E402

import concourse.bacc as bacc  # noqa: E402
import concourse.mybir as mybir  # noqa: E402
import concourse.tile as tile  # noqa: E402
from concourse.bass_utils import run_bass_kernel_spmd  # noqa: E402

B, N, C = 4, 4096, 128
NCORES = 8
NQ = N // 2  # queries per core
NB = 512  # query block size
NMT = N // C  # key tiles (32)
F32 = mybir.dt.float32
BF16 = mybir.dt.bfloat16
AOP = mybir.AluOpType
ACT = mybir.ActivationFunctionType

_cache: dict = {}
LAST_RESULTS = None


def _build(sc: float):
    nc = bacc.Bacc(None, target_bir_lowering=False, debug=True)

    xT = nc.declare_dram_parameter("xT", [C, N], BF16, isOutput=False)
    xN = nc.declare_dram_parameter("xN", [C, NMT * C], BF16, isOutput=False)
    Wp = nc.declare_dram_parameter("Wp", [C, 3 * C], BF16, isOutput=False)
    bp = nc.declare_dram_parameter("bp", [C, C + 1], F32, isOutput=False)
    y = nc.declare_dram_parameter("y", [NQ, C], F32, isOutput=True)

    with tile.TileContext(nc) as tc:
        with (
            tc.tile_pool(name="const", bufs=1) as cp,
            tc.tile_pool(name="ebuf", bufs=3) as ep,
            tc.tile_pool(name="esum", bufs=2) as esp,
            tc.tile_pool(name="small", bufs=2) as sp,
            tc.tile_pool(name="ybuf", bufs=3) as yp,
            tc.tile_pool(name="ps_s", bufs=2, space="PSUM") as ps_s,
            tc.tile_pool(name="ps_z", bufs=2, space="PSUM") as ps_z,
            tc.tile_pool(name="ps_rs", bufs=1, space="PSUM") as ps_rs,
            tc.tile_pool(name="ps_y", bufs=1, space="PSUM") as ps_y,
        ):
            xT_s = cp.tile([C, N], BF16)
            xN_s = cp.tile([C, NMT * C], BF16)
            wp_s = cp.tile([C, 3 * C], BF16)
            bp_s = cp.tile([C, C + 1], F32)
            ones_s = cp.tile([C, 1], BF16)
            kT_s = cp.tile([C, N], BF16)
            qT_s = cp.tile([C, NQ], BF16)
            wk_s = wp_s[:, C:2 * C]
            wvo_s = wp_s[:, 2 * C:3 * C]
            b2b_s = bp_s[:, 0:C]
            bq_s = bp_s[:, C:C + 1]

            nc.gpsimd.memset(ones_s[:], 1.0)
            nc.sync.dma_start(out=wp_s[:], in_=Wp[:])
            nc.sync.dma_start(out=bp_s[:], in_=bp[:])
            nc.sync.dma_start(out=xT_s[:, 0:NQ], in_=xT[:, 0:NQ])
            nc.sync.dma_start(out=xT_s[:, NQ:N], in_=xT[:, NQ:N])
            nc.scalar.dma_start(out=xN_s[:, 0:NQ], in_=xN[:, 0:NQ])
            nc.scalar.dma_start(out=xN_s[:, NQ:N], in_=xN[:, NQ:N])

            # projections, paired per [C, 1024] PSUM tile
            def proj_pair(w_ap, j):
                ps = ps_s.tile([C, 2 * NB], F32, tag="s", name="ps_proj")
                for g in range(2):
                    o = (2 * j + g) * NB
                    nc.tensor.matmul(
                        ps[:, g * NB:(g + 1) * NB], w_ap,
                        xT_s[:, o:o + NB], start=True, stop=True,
                    )
                return ps

            psq0 = proj_pair(wp_s[:, 0:C], 0)
            psk0 = proj_pair(wk_s, 0)
            psq1 = proj_pair(wp_s[:, 0:C], 1)
            psk1 = proj_pair(wk_s, 1)
            psk2 = proj_pair(wk_s, 2)
            psk3 = proj_pair(wk_s, 3)
            # q gets its bias; k bias cancels in softmax (copy only)
            nc.vector.tensor_scalar_add(qT_s[:, 0:2 * NB], psq0[:], bq_s)
            nc.vector.tensor_copy(kT_s[:, 0:2 * NB], psk0[:])
            nc.vector.tensor_scalar_add(qT_s[:, 2 * NB:4 * NB], psq1[:], bq_s)
            nc.vector.tensor_copy(kT_s[:, 2 * NB:4 * NB], psk1[:])
            nc.vector.tensor_copy(kT_s[:, 4 * NB:6 * NB], psk2[:])
            nc.vector.tensor_copy(kT_s[:, 6 * NB:8 * NB], psk3[:])

            pending_tail = [None]

            def make_tail(nb, es, z_ps, last):
                def emit():
                    esF = sp.tile([C, NB], BF16, tag="esF")
                    nc.vector.tensor_tensor(
                        esF[:], es[:, 0:NB], es[:, NB:2 * NB], op=AOP.add,
                    )
                    zT = sp.tile([C, NB], BF16, tag="zT")
                    if last:  # ScalarE is idle during the final tail
                        nc.scalar.copy(zT[:], z_ps[:])
                    else:
                        nc.vector.tensor_copy(zT[:], z_ps[:])
                    rs = ps_rs.tile([C, 4], F32, tag="rs")
                    for j in range(4):
                        nc.tensor.matmul(
                            rs[:, j:j + 1], esF[:, j * C:(j + 1) * C],
                            ones_s[:], start=True, stop=True,
                        )
                    rcp = sp.tile([C, 4], F32, tag="rcp")
                    nc.vector.reciprocal(rcp[:], rs[:])
                    for j in range(4):
                        pp = ps_y.tile([C, C], F32, tag="yp")
                        nc.tensor.matmul(
                            pp[:], zT[:, j * C:(j + 1) * C], wvo_s,
                            start=True, stop=True,
                        )
                        yt = yp.tile([C, C], F32, tag="yt")
                        nc.vector.scalar_tensor_tensor(
                            yt[:], pp[:], rcp[:, j:j + 1], b2b_s,
                            op0=AOP.mult, op1=AOP.add,
                        )
                        r0 = nb * NB + j * C
                        nc.sync.dma_start(out=y[r0:r0 + C, :], in_=yt[:])
                return emit

            for nb in range(NQ // NB):
                qsl = slice(nb * NB, (nb + 1) * NB)
                z_ps = ps_z.tile([C, NB], F32, tag="z")
                es = esp.tile([C, 2 * NB], BF16, tag="es")
                E_prev = None
                for mp in range(NMT // 2):
                    asl = slice(2 * mp * C, (2 * mp + 1) * C)
                    bsl = slice((2 * mp + 1) * C, (2 * mp + 2) * C)
                    pss = ps_s.tile([128, 2 * NB], F32, tag="s")
                    nc.tensor.matmul(
                        pss[:, 0:NB], kT_s[:, asl], qT_s[:, qsl],
                        start=True, stop=True,
                    )
                    nc.tensor.matmul(
                        pss[:, NB:2 * NB], kT_s[:, bsl], qT_s[:, qsl],
                        start=True, stop=True,
                    )
                    if mp == 2 and pending_tail[0] is not None:
                        pending_tail[0]()
                        pending_tail[0] = None
                    E = ep.tile([128, 2 * NB], BF16, tag="E")
                    nc.scalar.activation(E[:], pss[:], ACT.Exp, scale=sc)
                    nc.tensor.matmul(
                        z_ps[:], xN_s[:, asl], E[:, 0:NB],
                        start=(mp == 0), stop=False,
                    )
                    nc.tensor.matmul(
                        z_ps[:], xN_s[:, bsl], E[:, NB:2 * NB],
                        start=False, stop=(mp == NMT // 2 - 1),
                    )
                    if mp == 0:
                        E_prev = E
                    elif mp == 1:
                        nc.vector.tensor_tensor(
                            es[:], E_prev[:], E[:], op=AOP.add,
                        )
                        E_prev = None
                    else:
                        nc.vector.tensor_tensor(es[:], es[:], E[:], op=AOP.add)
                pending_tail[0] = make_tail(nb, es, z_ps, nb == NQ // NB - 1)
            pending_tail[0]()

    nc.compile()
    return nc


def kernel(x, W_fc, b_fc, W_out, b_out, scale):
    x = np.asarray(x, dtype=np.float32)
    W_fc = np.asarray(W_fc, dtype=np.float32)
    b_fc = np.asarray(b_fc, dtype=np.float32)
    W_out = np.asarray(W_out, dtype=np.float32)
    b_out = np.asarray(b_out, dtype=np.float32)
    scale = np.asarray(scale, dtype=np.float32)

    sc = float(1.0 / (math.sqrt(C) * float(scale[0])))
    key = ("v3", sc)
    if key not in _cache:
        _cache.clear()
        _cache[key] = _build(sc)
    nc = _cache[key]

    bf16 = ml_dtypes.bfloat16
    b2 = b_fc[2 * C:] @ W_out + b_out  # v-bias folded through the projection
    Wp = np.concatenate(
        [W_fc[:, :C], W_fc[:, C:2 * C], W_fc[:, 2 * C:] @ W_out], axis=1
    ).astype(bf16)
    bp = np.concatenate(
        [np.tile(b2, (C, 1)), b_fc[:C].reshape(C, 1)], axis=1
    ).astype(np.float32)
    common = {"Wp": np.ascontiguousarray(Wp), "bp": np.ascontiguousarray(bp)}
    in_maps = []
    for core in range(NCORES):
        b, h = core // 2, core % 2
        # rotate tokens so this core's queries are rows 0..NQ-1 (key order
        # inside the softmax sum is irrelevant)
        xb = np.roll(x[b], -h * NQ, axis=0) if h else x[b]
        xT_b = np.ascontiguousarray(xb.T).astype(bf16)
        # partition-major tiling: xN[p, mt*C + j] = x[mt*128 + p, j]
        xN_b = np.ascontiguousarray(
            xb.reshape(NMT, C, C).transpose(1, 0, 2).reshape(C, NMT * C)
        ).astype(bf16)
        in_maps.append({**common, "xT": xT_b, "xN": xN_b})

    res = run_bass_kernel_spmd(nc, in_maps, list(range(NCORES)))
    global LAST_RESULTS
    LAST_RESULTS = res

    yout = np.empty((B, N, C), dtype=np.float32)
    for core in range(NCORES):
        b, h = core // 2, core % 2
        yout[b, h * NQ:(h + 1) * NQ, :] = res.results[core]["y"]
    return yout


# revision 13
# speedup vs baseline: 1.6377x; 1.0449x over previous
"""Trainium2 Bass kernel for single-head attention (MDTA-style block).

Reference computation (per batch b, N=4096 tokens, C=128 channels):
    qkv = x @ W_fc + b_fc ; q,k,v = split(qkv)
    S   = (q @ k^T) / sqrt(C)
    A   = softmax(S / scale, axis=-1)
    out = (A @ v) @ W_out + b_out

Sharding: 8 cores = 4 batches x 2 query-halves (data parallel, no
cross-core comm). The token axis is rotated host-side for odd cores so
one SPMD program always sees its queries as tokens 0..2047 (softmax is
invariant to key order).

Per-core algorithm (v4 — flash-style, NxN never hits HBM):
  - q^T, k^T projections in [C, n] layout (bf16 PE matmuls from x^T).
    k-bias dropped (softmax shift-invariance), q-bias kept.
  - scores TRANSPOSED per 128-key tile: S^T[k,q] = kT.T @ qT into a
    [128, 1024] PSUM pair (2 key tiles), exp'd by ONE ScalarE
    activation; the exp stream (1 elem/lane/cycle) is the kernel's
    roofline, so everything else is arranged to keep it saturated.
  - value path via the identity (A@V)@Wo = (A@X) @ (Wv@Wo):
    Z = sum_mt x_mt^T @ E_mt accumulates with RAW x tiles stationary
    (no v projection at all); per 128-query chunk one matmul
    Z_chunk^T @ Wvo lands the result directly in [q, C] row layout.
  - softmax denominator: E pair-tiles accumulate on VectorE into one
    [128,1024] bf16 running sum (one wide add per pair), folded once;
    tiny matmuls esF_chunk^T @ ones give rowsums TRANSPOSED [q, 1] so
    the (8x slow) reciprocal runs on FD=4 only and the normalize is a
    per-partition scalar fused with the bias add.
  - latency hiding: input DMAs split over 4 engine queues (each queue
    streams ~60 GB/s with ~2us latency), projection PSUM lives in the
    tail-only banks so the score-pair double buffer is free from the
    first iteration, 8 zero matmuls warm the PE clock (HAM) during the
    DMA wait, and block tails are emitted two pair-iterations into the
    next block so the PE/ACT streams never drain.
"""

import math
import sys

import numpy as np

sys.path.insert(0, "/opt/trn_rl_repo")

import ml_dtypes  # noqa: E402

import concourse.bacc as bacc  # noqa: E402
import concourse.mybir as mybir  # noqa: E402
import concourse.tile as tile  # noqa: E402
from concourse.bass_utils import run_bass_kernel_spmd  # noqa: E402

B, N, C = 4, 4096, 128
NCORES = 8
NQ = N // 2  # queries per core
NB = 512  # query block size
NMT = N // C  # key tiles (32)
F32 = mybir.dt.float32
BF16 = mybir.dt.bfloat16
AOP = mybir.AluOpType
ACT = mybir.ActivationFunctionType

_cache: dict = {}
LAST_RESULTS = None


def _build(sc: float):
    nc = bacc.Bacc(None, target_bir_lowering=False, debug=True)

    xT = nc.declare_dram_parameter("xT", [C, N], BF16, isOutput=False)
    xN = nc.declare_dram_parameter("xN", [C, NMT * C], BF16, isOutput=False)
    Wp = nc.declare_dram_parameter("Wp", [C, 3 * C], BF16, isOutput=False)
    bp = nc.declare_dram_parameter("bp", [C, C + 1], F32, isOutput=False)
    y = nc.declare_dram_parameter("y", [NQ, C], F32, isOutput=True)

    with tile.TileContext(nc) as tc:
        with (
            tc.tile_pool(name="const", bufs=1) as cp,
            tc.tile_pool(name="ebuf", bufs=3) as ep,
            tc.tile_pool(name="esum", bufs=2) as esp,
            tc.tile_pool(name="small", bufs=2) as sp,
            tc.tile_pool(name="ybuf", bufs=3) as yp,
            tc.tile_pool(name="ps_s", bufs=2, space="PSUM") as ps_s,
            tc.tile_pool(name="ps_z", bufs=2, space="PSUM") as ps_z,
            tc.tile_pool(name="ps_rs", bufs=1, space="PSUM") as ps_rs,
            tc.tile_pool(name="ps_y", bufs=1, space="PSUM") as ps_y,
        ):
            xT_s = cp.tile([C, N], BF16)
            xN_s = cp.tile([C, NMT * C], BF16)
            wp_s = cp.tile([C, 3 * C], BF16)
            bp_s = cp.tile([C, C + 1], F32)
            ones_s = cp.tile([C, 1], BF16)
            warm_s = cp.tile([C, NB], BF16)
            kT_s = cp.tile([C, N], BF16)
            qT_s = cp.tile([C, NQ], BF16)
            wq_s = wp_s[:, 0:C]
            wk_s = wp_s[:, C:2 * C]
            wvo_s = wp_s[:, 2 * C:3 * C]
            b2b_s = bp_s[:, 0:C]
            bq_s = bp_s[:, C:C + 1]

            nc.gpsimd.memset(ones_s[:], 1.0)
            nc.gpsimd.memset(warm_s[:], 0.0)
            # inputs split over the 3 DMA-capable queues (sync/scalar/gpsimd)
            nc.scalar.dma_start(out=wp_s[:], in_=Wp[:])
            nc.scalar.dma_start(out=bp_s[:], in_=bp[:])
            for c in [0, 1, 2, 3, 6]:
                sl = slice(c * NB, (c + 1) * NB)
                nc.sync.dma_start(out=xT_s[:, sl], in_=xT[:, sl])
            for c in [4, 5, 7]:
                sl = slice(c * NB, (c + 1) * NB)
                nc.scalar.dma_start(out=xT_s[:, sl], in_=xT[:, sl])
            for g in range(4):
                sl = slice(g * 1024, (g + 1) * 1024)
                nc.gpsimd.dma_start(out=xN_s[:, sl], in_=xN[:, sl])

            # ~3.5us of dummy matmuls releases the HAM clock throttle while
            # the input DMAs stream in
            for i in range(8):
                wu = ps_s.tile([C, NB], F32, tag="s", name="wu")
                nc.tensor.matmul(
                    wu[:], warm_s[:, 0:C], warm_s[:], start=True, stop=True,
                )

            # projections: one [C, 512] PSUM tile each, alternating between
            # the two tail banks (which stay free until the first block tail)
            tailtag = [("rs", ps_rs), ("yp", ps_y)]
            proj_ctr = [0]

            def proj_half(w_ap, dst, col, dve, bias):
                tag, pool = tailtag[proj_ctr[0] % 2]
                proj_ctr[0] += 1
                ps = pool.tile([C, NB], F32, tag=tag, name="ps_proj")
                sl = slice(col, col + NB)
                nc.tensor.matmul(ps[:], w_ap, xT_s[:, sl], start=True, stop=True)
                if bias is not None:
                    dve.tensor_scalar_add(dst[:, sl], ps[:], bias)
                else:
                    dve.tensor_copy(dst[:, sl], ps[:])

            proj_half(wq_s, qT_s, 0 * NB, nc.vector, bq_s)
            proj_half(wk_s, kT_s, 0 * NB, nc.vector, None)
            proj_half(wq_s, qT_s, 1 * NB, nc.vector, bq_s)
            for j in range(1, 8):
                proj_half(wk_s, kT_s, j * NB, nc.vector, None)
            proj_half(wq_s, qT_s, 2 * NB, nc.vector, bq_s)
            proj_half(wq_s, qT_s, 3 * NB, nc.vector, bq_s)

            pending_tail = [None]

            def make_tail(nb, es, z_ps, last):
                def emit():
                    esF = sp.tile([C, NB], BF16, tag="esF")
                    nc.vector.tensor_tensor(
                        esF[:], es[:, 0:NB], es[:, NB:2 * NB], op=AOP.add,
                    )
                    zT = sp.tile([C, NB], BF16, tag="zT")
                    if last:  # ScalarE is idle during the final tail
                        nc.scalar.copy(zT[:], z_ps[:])
                    else:
                        nc.vector.tensor_copy(zT[:], z_ps[:])
                    rs = ps_rs.tile([C, 4], F32, tag="rs")
                    for j in range(4):
                        nc.tensor.matmul(
                            rs[:, j:j + 1], esF[:, j * C:(j + 1) * C],
                            ones_s[:], start=True, stop=True,
                        )
                    rcp = sp.tile([C, 4], F32, tag="rcp")
                    nc.vector.reciprocal(rcp[:], rs[:])
                    for j in range(4):
                        tag, pool = tailtag[(j + 1) % 2]
                        pp = pool.tile([C, C], F32, tag=tag, name="pp")
                        nc.tensor.matmul(
                            pp[:], zT[:, j * C:(j + 1) * C], wvo_s,
                            start=True, stop=True,
                        )
                        yt = yp.tile([C, C], F32, tag="yt")
                        nc.vector.scalar_tensor_tensor(
                            yt[:], pp[:], rcp[:, j:j + 1], b2b_s,
                            op0=AOP.mult, op1=AOP.add,
                        )
                        r0 = nb * NB + j * C
                        eng = nc.sync if j % 2 == 0 else nc.gpsimd
                        eng.dma_start(out=y[r0:r0 + C, :], in_=yt[:])
                return emit

            for nb in range(NQ // NB):
                qsl = slice(nb * NB, (nb + 1) * NB)
                z_ps = ps_z.tile([C, NB], F32, tag="z")
                es = esp.tile([C, 2 * NB], BF16, tag="es")
                E_prev = None
                for mp in range(NMT // 2):
                    asl = slice(2 * mp * C, (2 * mp + 1) * C)
                    bsl = slice((2 * mp + 1) * C, (2 * mp + 2) * C)
                    pss = ps_s.tile([128, 2 * NB], F32, tag="s")
                    nc.tensor.matmul(
                        pss[:, 0:NB], kT_s[:, asl], qT_s[:, qsl],
                        start=True, stop=True,
                    )
                    nc.tensor.matmul(
                        pss[:, NB:2 * NB], kT_s[:, bsl], qT_s[:, qsl],
                        start=True, stop=True,
                    )
                    if mp == 2 and pending_tail[0] is not None:
                        pending_tail[0]()
                        pending_tail[0] = None
                    E = ep.tile([128, 2 * NB], BF16, tag="E")
                    nc.scalar.activation(E[:], pss[:], ACT.Exp, scale=sc)
                    nc.tensor.matmul(
                        z_ps[:], xN_s[:, asl], E[:, 0:NB],
                        start=(mp == 0), stop=False,
                    )
                    nc.tensor.matmul(
                        z_ps[:], xN_s[:, bsl], E[:, NB:2 * NB],
                        start=False, stop=(mp == NMT // 2 - 1),
                    )
                    if mp == 0:
                        E_prev = E
                    elif mp == 1:
                        nc.vector.tensor_tensor(
                            es[:], E_prev[:], E[:], op=AOP.add,
                        )
                        E_prev = None
                    else:
                        nc.vector.tensor_tensor(es[:], es[:], E[:], op=AOP.add)
                pending_tail[0] = make_tail(nb, es, z_ps, nb == NQ // NB - 1)
            pending_tail[0]()

    nc.compile()
    return nc


def kernel(x, W_fc, b_fc, W_out, b_out, scale):
    x = np.asarray(x, dtype=np.float32)
    W_fc = np.asarray(W_fc, dtype=np.float32)
    b_fc = np.asarray(b_fc, dtype=np.float32)
    W_out = np.asarray(W_out, dtype=np.float32)
    b_out = np.asarray(b_out, dtype=np.float32)
    scale = np.asarray(scale, dtype=np.float32)

    sc = float(1.0 / (math.sqrt(C) * float(scale[0])))
    key = ("v4", sc)
    if key not in _cache:
        _cache.clear()
        _cache[key] = _build(sc)
    nc = _cache[key]

    bf16 = ml_dtypes.bfloat16
    b2 = b_fc[2 * C:] @ W_out + b_out  # v-bias folded through the projection
    Wp = np.concatenate(
        [W_fc[:, :C], W_fc[:, C:2 * C], W_fc[:, 2 * C:] @ W_out], axis=1
    ).astype(bf16)
    bpk = np.concatenate(
        [np.tile(b2, (C, 1)), b_fc[:C].reshape(C, 1)], axis=1
    ).astype(np.float32)
    common = {"Wp": np.ascontiguousarray(Wp), "bp": np.ascontiguousarray(bpk)}
    in_maps = []
    for core in range(NCORES):
        b, h = core // 2, core % 2
        # rotate tokens so this core's queries are rows 0..NQ-1 (key order
        # inside the softmax sum is irrelevant)
        xb = np.roll(x[b], -h * NQ, axis=0) if h else x[b]
        xT_b = np.ascontiguousarray(xb.T).astype(bf16)
        # partition-major tiling: xN[p, mt*C + j] = x[mt*128 + p, j]
        xN_b = np.ascontiguousarray(
            xb.reshape(NMT, C, C).transpose(1, 0, 2).reshape(C, NMT * C)
        ).astype(bf16)
        in_maps.append({**common, "xT": xT_b, "xN": xN_b})

    res = run_bass_kernel_spmd(nc, in_maps, list(range(NCORES)))
    global LAST_RESULTS
    LAST_RESULTS = res

    yout = np.empty((B, N, C), dtype=np.float32)
    for core in range(NCORES):
        b, h = core // 2, core % 2
        yout[b, h * NQ:(h + 1) * NQ, :] = res.results[core]["y"]
    return yout
